# revision 12
# baseline (speedup 1.0000x reference)
"""Cross-attention kernel for TRN2, 8 NeuronCores.

Sharding: core c -> (batch b = c//2, head-group g = c%2).  Each head-group is
8 heads = 512 of the 1024 d_model channels.  Within a core:
  QT = (Wq_g/8) @ q_b.T + bq_g/8        [512, 512]   (s, lq)   scale folded
  KT = Wk_g @ kv_b.T + bk_g             [512, 2048]  (s, lkv)
  V  = kv_b @ Wv_g.T + bv_g             [2048, 512]  (lkv, s)
  ST_h = Kh @ Qh.T                      [2048, 512]  per head (lkv, lq)
  PT_h = exp(ST_h)        (no max-sub: scores ~N(0,1), bounded)
  cT_h = V_h.T @ PT_h / colsum(PT_h)    [64, 512]    (s, lq)
  out_partial = cT.T.T @ Wo_g.T         [512, 1024]  (lq, d)
Host sums the two head-group partials per batch and adds bo.

All matmuls run as float32r (TF32-ish, full PE rate at N=512).  Head pairs are
packed onto the 128-wide PE array via partition-offset row/col tiling.
"""

import sys
if "/opt/trn_rl_repo" not in sys.path:
    sys.path.insert(0, "/opt/trn_rl_repo")

import numpy as np

import concourse.bass as bass
import concourse.mybir as mybir
import concourse.tile as tile
from concourse.bass_utils import run_bass_kernel_spmd

f32 = mybir.dt.float32
f32r = mybir.dt.float32r
EXP = mybir.ActivationFunctionType.Exp
IDENT = mybir.ActivationFunctionType.Identity

D = 1024        # d_model
S = 512         # per-core channel shard (8 heads x 64)
LQ = 512
LKV = 2048
CO = D // 128   # 8 contraction chunks
SO = S // 128   # 4 shard s-tiles
NT = LKV // 128  # 16 lkv tiles
NKC = LKV // 512  # 4 lkv 512-chunks


def _split_multi_waits(nc, max_waits=1):
    """This container's walrus allows only `max_waits` sync-wait commands per
    instruction; hoist the excess into standalone EventSemaphore insts."""
    ev_id = 0
    for f in nc.m.functions:
        for bb in f.blocks:
            new = []
            changed = False
            for inst in bb.instructions:
                si = inst.sync_info
                if si is not None and si.on_wait and len(si.on_wait) > max_waits:
                    waits = list(si.on_wait)
                    for sw in waits[:-max_waits]:
                        ev = mybir.InstEventSemaphore(
                            name=f"EVSPLIT-{ev_id}", engine=inst.engine,
                            sync_info=mybir.SyncInfo(on_wait=[sw], on_update=[]))
                        ev_id += 1
                        nc.register_instruction(ev, overwrite=True)
                        new.append(ev)
                    inst.sync_info = mybir.SyncInfo(
                        on_wait=waits[-max_waits:], on_update=list(si.on_update))
                    changed = True
                new.append(inst)
            if changed:
                bb.instructions = new
    return nc


def _build():
    nc = bass.Bass(trn_type="TRN2")

    # DRAM I/O (activations/weights pre-laid-out [128, outer, free] on host)
    qT = nc.dram_tensor("qT", [128, CO, LQ], f32r, kind="ExternalInput")
    kvT = nc.dram_tensor("kvT", [128, CO, LKV], f32r, kind="ExternalInput")
    wqT = nc.dram_tensor("wqT", [128, CO, S], f32r, kind="ExternalInput")
    wkT = nc.dram_tensor("wkT", [128, CO, S], f32r, kind="ExternalInput")
    wvT = nc.dram_tensor("wvT", [128, CO, S], f32r, kind="ExternalInput")
    woT = nc.dram_tensor("woT", [128, SO, D], f32r, kind="ExternalInput")
    bq = nc.dram_tensor("bq", [128, SO], f32, kind="ExternalInput")
    bk = nc.dram_tensor("bk", [128, SO], f32, kind="ExternalInput")
    bv = nc.dram_tensor("bv", [1, S], f32r, kind="ExternalInput")
    out = nc.dram_tensor("out", [SO, 128, D], f32, kind="ExternalOutput")

    with tile.TileContext(nc) as tc:
        with tc.tile_pool(name="wgt", bufs=1) as wgt, \
             tc.tile_pool(name="big", bufs=1) as big, \
             tc.tile_pool(name="strm", bufs=3) as strm, \
             tc.tile_pool(name="pt", bufs=2) as ptp, \
             tc.tile_pool(name="ostg", bufs=2) as ostg, \
             tc.tile_pool(name="sml", bufs=2) as sml, \
             tc.tile_pool(name="psA", bufs=4, space="PSUM") as psA, \
             tc.tile_pool(name="psC", bufs=2, space="PSUM") as psC:

            # ---- resident weights / constants ----
            wk_sb = wgt.tile([128, CO, S], f32r, name="wk_sb")
            wv_sb = wgt.tile([128, CO, S], f32r, name="wv_sb")
            kv_sb = wgt.tile([128, CO, LKV], f32r, name="kv_sb")
            bq_sb = wgt.tile([128, SO], f32, name="bq_sb")
            bk_sb = wgt.tile([128, SO], f32, name="bk_sb")
            bv_sb = wgt.tile([1, S], f32r, name="bv_sb")
            ones_f = wgt.tile([128, 128], f32, name="ones_f")
            ones = wgt.tile([128, 128], f32r, name="ones")
            for c in range(CO):
                nc.sync.dma_start(wk_sb[:, c, :], wkT[:, c, :])
                nc.sync.dma_start(wv_sb[:, c, :], wvT[:, c, :])
                nc.sync.dma_start(kv_sb[:, c, :], kvT[:, c, :])
            nc.sync.dma_start(bq_sb, bq[:])
            nc.sync.dma_start(bk_sb, bk[:])
            nc.sync.dma_start(bv_sb, bv[:])
            nc.vector.memset(ones_f, 1.0)
            nc.vector.tensor_copy(ones, ones_f)

            # broadcast bv across partitions: ones[1,128].T @ bv[1,512]
            bv_ps = psA.tile([128, S], f32, name="bv_ps", tag="mm")
            nc.tensor.matmul(bv_ps, ones[0:1, :], bv_sb, start=True, stop=True)
            bv_bc = wgt.tile([128, S], f32r, name="bv_bc")
            nc.vector.tensor_copy(bv_bc, bv_ps)

            # ---- resident intermediates ----
            KT_sb = big.tile([128, SO, LKV], f32r, name="KT_sb")   # (s, lkv)
            # V padded per head with a ones column: [lkv, t, head, 64+1].
            # The ones column makes attn@V also produce the softmax
            # denominator as psum row 64 (col-tiling to upper partitions is
            # rejected by this walrus, so no separate denominator matmuls).
            Vp_sb = big.tile([128, NT, 8, 65], f32r, name="Vp_sb")
            QT_sb = big.tile([128, SO, LQ], f32r, name="QT_sb")    # (s, lq)
            cT_sb = big.tile([128, SO, LQ], f32r, name="cT_sb")    # (s, lq)
            nc.vector.tensor_copy(
                Vp_sb[:, :, :, 64:65],
                ones_f[:, 0:128].rearrange("p (a b c) -> p a b c", a=NT, b=8, c=1))

            # ---- K projection: KT[s, lkv] += wk[c,s].T @ kv_sb[c, lkv] ----
            for ch in range(NKC):
                kps = [psA.tile([128, 512], f32, name=f"kps{o}_{ch}", tag="mm")
                       for o in range(SO)]
                for c in range(CO):
                    for o in range(SO):
                        nc.tensor.matmul(
                            kps[o], wk_sb[:, c, o * 128:(o + 1) * 128],
                            kv_sb[:, c, ch * 512:(ch + 1) * 512],
                            start=(c == 0), stop=(c == CO - 1))
                for o in range(SO):
                    nc.scalar.activation(
                        KT_sb[:, o, ch * 512:(ch + 1) * 512], kps[o], IDENT,
                        bias=bk_sb[:, o:o + 1])

            # ---- V projection: V[lkv, s] += kv_sb[c, lkv].T @ wv[c, s] ----
            for t in range(NT):
                vps = psA.tile([128, 512], f32, name="vps", tag="mm")
                for c in range(CO):
                    nc.tensor.matmul(vps, kv_sb[:, c, t * 128:(t + 1) * 128],
                                     wv_sb[:, c, :],
                                     start=(c == 0), stop=(c == CO - 1))
                nc.vector.tensor_add(
                    Vp_sb[:, t, :, 0:64],
                    vps.rearrange("p (h d) -> p h d", h=8),
                    bv_bc.rearrange("p (h d) -> p h d", h=8))

            # ---- Q projection: QT[s, lq] += wq[c,s].T @ qT[c, lq] ----
            qps = [psA.tile([128, 512], f32, name=f"qps{o}", tag="mm")
                   for o in range(SO)]
            for c in range(CO):
                qtt = strm.tile([128, 512], f32r, name="qtt", tag="st512")
                nc.sync.dma_start(qtt, qT[:, c, :])
                wqc = strm.tile([128, S], f32r, name="wqc", tag="st512")
                nc.sync.dma_start(wqc, wqT[:, c, :])
                for o in range(SO):
                    nc.tensor.matmul(
                        qps[o], wqc[:, o * 128:(o + 1) * 128], qtt,
                        start=(c == 0), stop=(c == CO - 1))
            for o in range(SO):
                nc.scalar.activation(QT_sb[:, o, :], qps[o], IDENT,
                                     bias=bq_sb[:, o:o + 1])

            # ---- attention, head pairs (2o, 2o+1) ----
            # scores row-tiled (dh=64 contraction at row offsets 0/64);
            # attn@V per head with M=65 (64 V cols + ones col -> denominator
            # lands in psum row 64).
            for o in range(SO):
                ctxA = psC.tile([65, 512], f32, name="ctxA", tag="ctxA")
                ctxB = psC.tile([65, 512], f32, name="ctxB", tag="ctxB")
                for t in range(NT):
                    stA = psA.tile([128, 512], f32, name="stA", tag="mm")
                    stB = psA.tile([128, 512], f32, name="stB", tag="mm")
                    # S.T tile = Kh[., t-slice].T-contraction over dh=64 rows
                    nc.tensor.matmul(stA, KT_sb[0:64, o, t * 128:(t + 1) * 128],
                                     QT_sb[0:64, o, :], start=True, stop=True)
                    nc.tensor.matmul(stB, KT_sb[64:128, o, t * 128:(t + 1) * 128],
                                     QT_sb[64:128, o, :], start=True, stop=True)
                    ptA = ptp.tile([128, 512], f32r, name="ptA", tag="ptA")
                    ptB = ptp.tile([128, 512], f32r, name="ptB", tag="ptB")
                    nc.scalar.activation(ptA, stA, EXP)
                    nc.scalar.activation(ptB, stB, EXP)
                    st = (t == 0)
                    sp = (t == NT - 1)
                    nc.tensor.matmul(ctxA, Vp_sb[:, t, 2 * o, :], ptA,
                                     start=st, stop=sp)
                    nc.tensor.matmul(ctxB, Vp_sb[:, t, 2 * o + 1, :], ptB,
                                     start=st, stop=sp)
                # normalize: cT_h = ctx_h[0:64] * (1/ctx_h[64]) bcast to 64 rows
                for h, ctx in ((0, ctxA), (1, ctxB)):
                    rc = sml.tile([1, 512], f32r, name="rc", tag="rc")
                    with nc.allow_low_precision(reason="softmax recip f32r"):
                        nc.vector.reciprocal(rc, ctx[64:65, :])
                    nb_ps = psA.tile([64, 512], f32, name="nb_ps", tag="mm")
                    nc.tensor.matmul(nb_ps, ones[0:1, 0:64], rc,
                                     start=True, stop=True)
                    nb_sb = sml.tile([64, 512], f32, name="nb_sb", tag="nb")
                    nc.vector.tensor_copy(nb_sb, nb_ps)
                    nc.vector.tensor_mul(
                        cT_sb[h * 64:(h + 1) * 64, o, :], ctx[0:64, :], nb_sb)

            # ---- out projection: out[lq, d] += cT[s, lq-slice].T @ wo[s, d] ----
            for dc in range(2):
                opss = [psA.tile([128, 512], f32, name=f"ops{lt}", tag="mm")
                        for lt in range(SO)]
                for o in range(SO):
                    woc = strm.tile([128, 512], f32r, name="woc", tag="st512")
                    nc.sync.dma_start(woc, woT[:, o, dc * 512:(dc + 1) * 512])
                    for lt in range(SO):
                        nc.tensor.matmul(
                            opss[lt], cT_sb[:, o, lt * 128:(lt + 1) * 128],
                            woc, start=(o == 0), stop=(o == SO - 1))
                for lt in range(SO):
                    ot = ostg.tile([128, 512], f32, name="ot", tag="ot")
                    nc.vector.tensor_copy(ot, opss[lt])
                    nc.sync.dma_start(out[lt, :, dc * 512:(dc + 1) * 512], ot)

    return _split_multi_waits(nc)


_NC = None


def _get_nc():
    global _NC
    if _NC is None:
        _NC = _build()
    return _NC


def _shard(q, kv, Wq, bq, Wk, bk, Wv, bv, Wo, bo):
    def lay(a2d, co):  # [co*128, F] -> [128, co, F]
        F = a2d.shape[1]
        return np.ascontiguousarray(
            a2d.reshape(co, 128, F).transpose(1, 0, 2))

    in_maps = []
    for core in range(8):
        b, g = core // 2, core % 2
        sl = slice(g * S, (g + 1) * S)
        m = {
            "qT": lay(np.ascontiguousarray(q[b].T), CO),
            "kvT": lay(np.ascontiguousarray(kv[b].T), CO),
            "wqT": lay(np.ascontiguousarray((Wq[sl] * 0.125).T), CO),
            "wkT": lay(np.ascontiguousarray(Wk[sl].T), CO),
            "wvT": lay(np.ascontiguousarray(Wv[sl].T), CO),
            "woT": lay(np.ascontiguousarray(Wo[:, sl].T), SO),
            "bq": np.ascontiguousarray((bq[sl] * 0.125).reshape(SO, 128).T),
            "bk": np.ascontiguousarray(bk[sl].reshape(SO, 128).T),
            "bv": np.ascontiguousarray(bv[sl].reshape(1, S)),
        }
        in_maps.append({k: v.astype(np.float32, copy=False) for k, v in m.items()})
    return in_maps


def _run(in_maps, trace=False):
    res = run_bass_kernel_spmd(_get_nc(), in_maps, core_ids=list(range(8)),
                               trace=trace)
    return res


def kernel(q, kv, Wq, bq, Wk, bk, Wv, bv, Wo, bo, _trace=False):
    q, kv = np.asarray(q, np.float32), np.asarray(kv, np.float32)
    Wq, Wk = np.asarray(Wq, np.float32), np.asarray(Wk, np.float32)
    Wv, Wo = np.asarray(Wv, np.float32), np.asarray(Wo, np.float32)
    bq, bk = np.asarray(bq, np.float32), np.asarray(bk, np.float32)
    bv, bo = np.asarray(bv, np.float32), np.asarray(bo, np.float32)

    in_maps = _shard(q, kv, Wq, bq, Wk, bk, Wv, bv, Wo, bo)
    res = _run(in_maps, trace=_trace)
    B = q.shape[0]
    outp = np.empty((B, LQ, D), np.float32)
    for b in range(B):
        p0 = res.results[2 * b]["out"].reshape(LQ, D)
        p1 = res.results[2 * b + 1]["out"].reshape(LQ, D)
        outp[b] = p0 + p1 + bo[None, :]
    if _trace:
        kernel._last_exec_ns = res.exec_time_ns
        kernel._last_trace = res.instructions_and_trace
    return outp
